# revision 27
# baseline (speedup 1.0000x reference)
"""Trainium2 Bass kernel for nn_MultiHeadCrossAttention_84542136254903.

Sliding-window causal cross-attention (query i attends keys [i-256, i]),
16 heads, d_model 1024. Sharded data-parallel over batch B=8 across the 8
NeuronCores; each core runs the full per-batch-element pipeline:

  q = query @ Wq.T + bq ; k = key @ Wk.T + bk ; v = value @ Wv.T + bv
  S = (q k^T) / 8  (banded: |i-j| window)  ;  P = softmax_masked(S)
  out = (P v) @ Wo.T + bo

Layout strategy (per core):
  - query/key/value and all weights are transposed on the host (cheap numpy
    marshalling, like the batch sharding itself) and converted to bf16, so
    SBUF holds query^T, key^T, value^T, Wq^T, Wk^T, Wv^T, Wo^T via plain
    contiguous DMA loads at half the HBM traffic of fp32. All loads go
    through the two HWDGE queues (sync + scalar), V path first since it
    gates every PV matmul; descriptor processing is ~630ns per DMA so the
    tiny constant loads are emitted behind the V path.
  - Projections run in bf16 (full rate on the PE, FWL weight loads) and
    produce QT=[d_model, Q] and KT=[d_model, T] (feature-major) plus V in
    natural [T, d_model] bf16 with a per-head ones column appended.
  - Attention is computed transposed: for each (head, key-chunk of 128),
    ST[j, i] over the 384-wide query window [j0, j0+384), with the two heads
    of a pair row-packed via tile_position so their K=64 matmuls run
    concurrently in disjoint row-groups of the PE array. exp on ACT ->
    bf16, band mask as a bf16 multiply (split between DVE and Pool engines;
    Pool cannot touch PSUM so SBUF-only work like this is all it can take),
    then bf16 PV matmuls accumulate OT_aug[65, 1024] per head in PSUM via
    overlapping-window accumulation (per-2KB-region pending-zero semantics).
    Row 64 (from the ones column of V) is the softmax denominator, already
    in free-dim layout: DVE reciprocal (PSUM row -> bf16 SBUF row), a
    stride-0 DMA replicates it across 64 partitions off-engine (cheaper
    than Pool partition_broadcast), and one fused DVE multiply normalizes
    and evacuates PSUM->SBUF.
  - Out-projection in bf16 reads OT directly (both operands feature-major,
    no transposes anywhere on the PE), stages PSUM->SBUF (DMA cannot read
    PSUM) and DMAs to DRAM.
"""

import os
import numpy as np

import concourse.bass as bass
import concourse.bacc as bacc
import concourse.tile as tile
from concourse import mybir
from concourse.bass_utils import run_bass_kernel_spmd
from concourse.vector_clock import ScopedClock
from contextlib import ExitStack

F32 = mybir.dt.float32
F32R = mybir.dt.float32r
BF16 = mybir.dt.bfloat16
AF = mybir.ActivationFunctionType

B, Q, T = 8, 1024, 1024
DQ, DK, DV, DM, H = 128, 256, 256, 1024, 16
HD = DM // H  # 64
WIN = 512
SCALE = HD ** -0.5
N_CORES = 8
NCH = T // 128  # 8 key chunks / query chunks / m chunks

# matmul dtype for the fp32 stages (projections, scores, out-proj).
MM_DT = F32R

# head-pairs whose band-mask multiply runs on the Pool (gpsimd) engine
# instead of DVE, to balance engine load. (Pool cannot touch PSUM, so the
# mask multiply on SBUF pt tiles is the main work it can absorb.)
POOL_MASK_PAIRS = frozenset({3, 4, 5, 6, 7})


class _TileContextFixed(tile.TileContext):
    """Work around this walrus build's 1-sem-wait-per-CTRL-instruction limit:
    the Tile kernel-tail drain arrives with one wait per outstanding
    semaphore; keep the first on the Drain and chain the rest as single-wait
    nops on the same engine (sequential, so semantics are unchanged)."""

    def _drain_and_barrier(self, tick_clock, wait_clock):
        nc = self.nc
        drain_inst = nc.sync.drain()
        wait_clock.add_sem_waits(
            drain_inst.ins, ScopedClock({None: tick_clock.global_clock})
        )
        si = drain_inst.ins.sync_info
        if si is not None and si.on_wait and len(si.on_wait) > 1:
            waits = list(si.on_wait)
            si.on_wait = [waits[0]]
            drain_inst.ins.sync_info = si
            sem_map = {s.name: s for s in self.sems.allocated().values()}
            for w in waits[1:]:
                sem = sem_map[w.ant_name]
                assert w.wait_mode == "sem-ge-imm", w.wait_mode
                nc.sync.wait_ge(sem, w.wait_value)

        nc.all_engine_barrier()
        assert self.sems is not None
        popped = nc._tile_sem_poison_stack.pop()
        assert popped is self._sem_poison
        nc.clear_and_free_semaphores(list(self.sems.allocated().values()))
        nc.all_engine_barrier()


def _win(c):
    """Query window width for key chunk c (keys [128c, 128c+128)).

    Chunks 0 and 4 are widened to 512 so their PT block fully covers a
    512-column PSUM region of OT_aug: the widened area is entirely masked
    (i - j > 256 there), and it lets the first PV matmul per region open it
    with start=True covering the whole pending-zero region."""
    if c in (0, 4):
        return 512
    return min(384, T - 128 * c)


def build_nc(repeat=1):
    nc = bacc.Bacc(
        "TRN2", target_bir_lowering=False, debug=False, num_devices=N_CORES
    )

    def din(name, shape, dt=F32):
        return nc.dram_tensor(name, shape, dt, kind="ExternalInput").ap()

    qT_d = nc.dram_tensor("qT", [DQ, Q], BF16, kind="ExternalInput").ap()
    kT_d = nc.dram_tensor("kT", [DK, T], BF16, kind="ExternalInput").ap()
    vT_d = nc.dram_tensor("vT", [DV, T], BF16, kind="ExternalInput").ap()
    WqT_d = nc.dram_tensor("WqT", [DQ, DM], BF16, kind="ExternalInput").ap()
    WkT_d = nc.dram_tensor("WkT", [DK, DM], BF16, kind="ExternalInput").ap()
    WvT_d = nc.dram_tensor("WvT", [DV, DM], BF16, kind="ExternalInput").ap()
    WoT_d = nc.dram_tensor("WoT", [DM, DM], BF16, kind="ExternalInput").ap()
    bq_l = din("bq_l", [128, NCH])       # bq_l[p, c] = bq[128c + p]
    bk_l = din("bk_l", [128, NCH])
    bv_row = din("bv_row", [1, DM], F32R)
    bo_row = din("bo_row", [1, DM], F32R)
    ones_row = din("ones_row", [1, 128], F32R)
    mask01 = nc.dram_tensor("mask01", [128, 512], BF16, kind="ExternalInput").ap()

    out = nc.dram_tensor("out", [Q, DM], F32, kind="ExternalOutput").ap()

    with _TileContextFixed(nc) as tc, ExitStack() as ctx:
        small = ctx.enter_context(tc.tile_pool(name="small", bufs=1))
        persist = ctx.enter_context(tc.tile_pool(name="persist", bufs=1))

        # ---- small constant tiles (loads emitted below, after the V path) ----
        bq_t = small.tile([128, NCH], F32, tag="bq")
        bk_t = small.tile([128, NCH], F32, tag="bk")
        bv_t = small.tile([1, DM], F32R, tag="bv")
        bo_t = small.tile([1, DM], F32R, tag="bo")
        ones_t = small.tile([1, 128], F32R, tag="ones")
        mask_t = small.tile([128, 512], BF16, tag="mask")

        # ---- persistent tiles ------------------------------------------------
        qT = persist.tile([128, Q], BF16, tag="qT")          # query^T [DQ, Q]
        WqT = persist.tile([128, DM], BF16, tag="WqT")       # Wq^T [DQ, DM]
        kT = [persist.tile([128, T], BF16, tag=f"kT{i}", name=f"kT{i}") for i in range(2)]
        WkT = [persist.tile([128, DM], BF16, tag=f"WkT{i}", name=f"WkT{i}") for i in range(2)]
        WoT = [persist.tile([128, DM], BF16, tag=f"WoT{i}", name=f"WoT{i}") for i in range(NCH)]
        QT = [persist.tile([128, Q], BF16, tag=f"QT{i}", name=f"QT{i}") for i in range(NCH)]
        KT = [persist.tile([128, T], BF16, tag=f"KT{i}", name=f"KT{i}") for i in range(NCH)]
        # V natural [T, d_model] bf16, 65 columns per head (64 dims + ones)
        Vb = [persist.tile([128, 65 * H], BF16, tag=f"Vb{i}", name=f"Vb{i}") for i in range(NCH)]
        OT = [persist.tile([128, Q], BF16, tag=f"OT{i}", name=f"OT{i}") for i in range(NCH)]

        # ---- loads (V path first: it gates every PV matmul) ------------------
        # All loads go through the two HWDGE queues (sync + scalar); SWDGE
        # (gpsimd et al) burns Pool engine time per descriptor. Order is
        # dependency-critical-path: V path, then K, Q, WoT last.
        vw = ctx.enter_context(tc.tile_pool(name="vw", bufs=1))
        vT = [vw.tile([128, T], BF16, tag=f"vT{i}", name=f"vT{i}") for i in range(2)]
        WvT = [vw.tile([128, DM], BF16, tag=f"WvT{i}", name=f"WvT{i}") for i in range(2)]
        for i in range(2):
            nc.sync.dma_start(vT[i][:], vT_d[128 * i:128 * (i + 1), :])
            nc.scalar.dma_start(WvT[i][:], WvT_d[128 * i:128 * (i + 1), :])
        # consts the V path needs (bias matmul) right after vT on sync;
        # the rest follow on scalar behind WvT.
        nc.sync.dma_start(ones_t[:], ones_row[:])
        nc.sync.dma_start(bv_t[:], bv_row[:])
        nc.scalar.dma_start(mask_t[:], mask01[:])
        nc.scalar.dma_start(bq_t[:], bq_l[:])
        nc.scalar.dma_start(bk_t[:], bk_l[:])
        nc.scalar.dma_start(bo_t[:], bo_row[:])
        for i in range(2):
            nc.sync.dma_start(kT[i][:], kT_d[128 * i:128 * (i + 1), :])
            nc.scalar.dma_start(WkT[i][:], WkT_d[128 * i:128 * (i + 1), :])
        nc.sync.dma_start(qT[:], qT_d[:])
        nc.scalar.dma_start(WqT[:], WqT_d[:])
        for i in range(NCH):
            eng = (nc.scalar, nc.sync)[i % 2]
            eng.dma_start(WoT[i][:], WoT_d[128 * i:128 * (i + 1), :])

        # PV window pieces, split at the 512-col PSUM region boundary.
        # Chunks 0 and 4 have 512-wide windows, so the first piece of each
        # region covers it fully (opens it with start=True).
        pieces = []  # (c, lo, hi, region)
        for c in range(NCH):
            lo, hi = 128 * c, 128 * c + _win(c)
            for b0, b1 in ((0, 512), (512, 1024)):
                ps_, pe_ = max(lo, b0), min(hi, b1)
                if ps_ < pe_:
                    pieces.append((c, ps_, pe_, b0 // 512))
        pieces.sort(key=lambda p: (p[3], p[2] - p[1] != 512, p[1]))
        first_i, last_i = {}, {}
        for idx, (c, ps_, pe_, rg) in enumerate(pieces):
            first_i.setdefault(rg, idx)
            last_i[rg] = idx

        for _rep in range(repeat):
            # ---- V projection (gates every PV matmul) ------------------------
            # Own 4-buf PSUM scope so it runs mm-paced; closes before the
            # pipelined middle opens its pools.
            with tc.tile_pool(name="vproj_psum", bufs=4, space="PSUM") as pjv:
                for jc in range(NCH):
                    js = slice(128 * jc, 128 * (jc + 1))
                    for half in range(2):
                        ps = pjv.tile([128, 512], F32, tag="vp")
                        sl = slice(512 * half, 512 * (half + 1))
                        for cc in range(2):
                            nc.tensor.matmul(
                                ps[:],
                                vT[cc][:, js],
                                WvT[cc][:, sl],
                                start=(cc == 0), stop=False,
                            )
                        nc.tensor.matmul(
                            ps[:], ones_t[:], bv_t[:, sl],
                            start=False, stop=True,
                        )
                        vdst = Vb[jc][:].rearrange("p (h c) -> p h c", c=65)
                        vsrc = ps[:].rearrange("p (h c) -> p h c", c=64)
                        h0 = 8 * half
                        if half == 0:
                            nc.vector.tensor_copy(vdst[:, h0:h0 + 8, 0:64], vsrc[:])
                        else:
                            nc.scalar.activation(
                                vdst[:, h0:h0 + 8, 0:64], vsrc[:], AF.Copy,
                            )
                    ones_col = Vb[jc][:].rearrange("p (h c) -> p h c", c=65)[:, :, 64:65]
                    nc.gpsimd.memset(ones_col, 1.0)

            # PSUM budget for the pipelined middle: proj/outproj pool 2 banks
            # + st 2 banks + ot 4 banks = 8. Pools coexist so the Q/K
            # projections software-pipeline into the attention loop and PE
            # fills exp-wait gaps with the next pair's projections.
            with (
                tc.tile_pool(name="proj_psum", bufs=2, space="PSUM") as pj,
                tc.tile_pool(name="st_psum", bufs=2, space="PSUM") as stp,
                tc.tile_pool(name="ot_psum", bufs=2, space="PSUM") as otp,
                tc.tile_pool(name="pt_sb", bufs=10) as ptp,
                tc.tile_pool(name="recip_sb", bufs=4) as rcp,
            ):

                # Q/K projection for one pair, split into 4 emission steps so
                # they interleave between score chunks of the previous pair.
                # Bias-add evacuation on DVE (ACT must stay exp-only in the
                # pipelined middle phase).
                def kq_steps(hp):
                    def q_half(half):
                        def emit():
                            sl = slice(512 * half, 512 * (half + 1))
                            ps = pj.tile([128, 512], F32, tag="pp")
                            nc.tensor.matmul(
                                ps[:], WqT[:, 128 * hp:128 * (hp + 1)], qT[:, sl],
                                start=True, stop=True,
                            )
                            nc.vector.tensor_scalar_add(
                                QT[hp][:, sl], ps[:], bq_t[:, hp:hp + 1],
                            )
                        return emit
                    def k_half(half):
                        def emit():
                            sl = slice(512 * half, 512 * (half + 1))
                            ps = pj.tile([128, 512], F32, tag="pp")
                            for cc in range(2):
                                nc.tensor.matmul(
                                    ps[:],
                                    WkT[cc][:, 128 * hp:128 * (hp + 1)],
                                    kT[cc][:, sl],
                                    start=(cc == 0), stop=(cc == 1),
                                )
                            nc.vector.tensor_scalar_add(
                                KT[hp][:, sl], ps[:], bk_t[:, hp:hp + 1],
                            )
                        return emit
                    return [q_half(0), k_half(0), q_half(1), k_half(1)]

                def emit_chunk(hp, c, pts):
                    heads = (2 * hp, 2 * hp + 1)
                    W = _win(c)          # PV window (512 for c in {0,4})
                    Wc = min(W, 384)     # live score columns
                    i0 = 128 * c
                    st = stp.tile([128, 1024], F32, tag="st")
                    for h in heads:
                        prow = (h % 2) * 64
                        o = 512 * (h % 2)
                        # tile_position row-packs the two heads' K=64 matmuls
                        # into disjoint row-groups of the PE array so they
                        # run concurrently.
                        nc.tensor.matmul(
                            st[:, o:o + Wc],
                            KT[hp][prow:prow + 64, 128 * c:128 * (c + 1)],
                            QT[hp][prow:prow + 64, i0:i0 + Wc],
                            start=True, stop=True,
                            tile_position=(prow, 0),
                        )
                    pt = ptp.tile([128, 1024], BF16, tag="pt")
                    st3 = st[:].rearrange("p (g f) -> p g f", g=2)
                    pt3 = pt[:].rearrange("p (g f) -> p g f", g=2)
                    nc.scalar.activation(
                        pt3[:, :, 0:Wc], st3[:, :, 0:Wc], AF.Exp,
                        scale=float(SCALE),
                    )
                    # Only the first and third 128-col blocks of each window
                    # are partially masked; one strided multiply covers both
                    # heads. Pool (SBUF-only engine, ~2.3x slower at TT)
                    # takes four chunks, DVE the rest, balancing the middle.
                    eng = nc.gpsimd if c in (0, 1, 2, 4) else nc.vector
                    pt4 = pt[:].rearrange("p (g b f) -> p g b f", g=2, f=128)
                    m3 = mask_t[:].rearrange("p (b f) -> p b f", f=128)
                    if Wc > 256:
                        v = pt4[:, :, 0:3:2, :]
                        m = m3[:, 0:3:2, :]
                    else:
                        v = pt4[:, :, 0:1, :]
                        m = m3[:, 0:1, :]
                    # broadcast the mask across the two head-halves with a
                    # stride-0 free dim
                    m2 = bass.AP(
                        m.tensor, m.offset,
                        [m.ap[0], [0, 2]] + list(m.ap[1:]),
                    )
                    eng.tensor_mul(v, v, m2)
                    if W > Wc:
                        nc.gpsimd.memset(pt3[:, :, Wc:W], 0.0)
                    pts.append(pt)

                def emit_pv_norm(hp, pts):
                    # PV + normalize per (head, 512-col PSUM region): region
                    # tiles are one bank each, so st can keep 2 bufs and the
                    # whole middle fits the 8-bank PSUM budget.
                    heads = (2 * hp, 2 * hp + 1)
                    for rg in (0, 1):
                        for h in heads:
                            prow = (h % 2) * 64
                            o = 512 * (h % 2)
                            ot = otp.tile([65, 512], F32, tag="ot")
                            rp = [p for p in pieces if p[3] == rg]
                            for idx, (c, ps_, pe_, _rg) in enumerate(rp):
                                nc.tensor.matmul(
                                    ot[:, ps_ - 512 * rg:pe_ - 512 * rg],
                                    Vb[c][:, 65 * h:65 * (h + 1)],
                                    pts[c][:, o + ps_ - 128 * c:o + pe_ - 128 * c],
                                    start=(idx == 0),
                                    stop=(idx == len(rp) - 1),
                                    skip_group_check=True,
                                )
                            # normalize: row 64 holds the softmax
                            # denominators. DVE reciprocal (PSUM row -> bf16
                            # SBUF row; reciprocal_approx_fast would be 5x
                            # cheaper but its custom-DVE op miscomputes on HW
                            # through this NEFF path), stride-0 DMA
                            # replicates it across 64 partitions
                            # (off-engine), DVE multiply normalizes and
                            # evacuates PSUM->SBUF.
                            recipT = rcp.tile([1, 512], BF16, tag="recipT")
                            with nc.allow_low_precision(
                                reason="softmax denom recip bf16; tol 2e-2"
                            ):
                                nc.vector.reciprocal(recipT[:], ot[64:65, :])
                            recipB = rcp.tile([64, 512], BF16, tag="recipB")
                            srcT = recipT[:]
                            src0 = bass.AP(
                                srcT.tensor, srcT.offset,
                                [srcT.ap[0], [0, 64]] + list(srcT.ap[1:]),
                            )
                            nc.sync.dma_start(recipB[:], src0)
                            nc.vector.tensor_mul(
                                OT[hp][prow:prow + 64, 512 * rg:512 * (rg + 1)],
                                ot[0:64, :], recipB[:],
                            )

                # ---- pipelined attention middle ------------------------------
                # Per pair: emit chunk 0, then the previous pair's PV+normalize
                # (so PE works while ACT streams this pair's exps), remaining
                # chunks with the next pair's K/Q projection steps interleaved
                # into the exp-wait gaps.
                for step in kq_steps(0):
                    step()
                prev = None
                for hp in range(H // 2):
                    steps = kq_steps(hp + 1) if hp + 1 < H // 2 else []
                    pts = []
                    for c in range(NCH):
                        emit_chunk(hp, c, pts)
                        if c == 0 and prev is not None:
                            emit_pv_norm(*prev)
                        if c in (1, 3, 5, 7) and steps:
                            steps[(c - 1) // 2]()
                    prev = (hp, pts)
                emit_pv_norm(*prev)

            # ---- out projection (own scope: middle PSUM pools are closed,
            # so 4 psum bufs keep PE mm-paced; out DMAs alternate the two
            # HWDGE queues so the drain isn't single-queue bound) ------------
            with (
                tc.tile_pool(name="out_psum", bufs=4, space="PSUM") as op,
                tc.tile_pool(name="out_sb", bufs=6) as osb,
            ):
                for ic in range(NCH):
                    isl = slice(128 * ic, 128 * (ic + 1))
                    for half in range(2):
                        ps = op.tile([128, 512], F32, tag="op")
                        sl = slice(512 * half, 512 * (half + 1))
                        for kc in range(NCH):
                            nc.tensor.matmul(
                                ps[:],
                                OT[kc][:, isl],
                                WoT[kc][:, sl],
                                start=(kc == 0), stop=False,
                            )
                        nc.tensor.matmul(
                            ps[:], ones_t[:], bo_t[:, sl],
                            start=False, stop=True,
                        )
                        st_out = osb.tile([128, 512], F32, tag="ostage")
                        if half == 0:
                            nc.vector.tensor_copy(st_out[:], ps[:])
                        else:
                            nc.scalar.copy(st_out[:], ps[:])
                        eng = (nc.sync, nc.scalar)[(2 * ic + half) % 2]
                        eng.dma_start(out[isl, sl], st_out[:])

    nc.compile()
    return nc


_NC_CACHE = None


def _host_inputs(inputs):
    """Per-core in_maps from the full-batch inputs (host-side transposes)."""
    def f32(name):
        return np.asarray(inputs[name], dtype=np.float32)

    import ml_dtypes as _mld
    BF = _mld.bfloat16
    q, k, v = f32("query"), f32("key"), f32("value")
    WqT = np.ascontiguousarray(f32("Wq").T).astype(BF)
    WkT = np.ascontiguousarray(f32("Wk").T).astype(BF)
    WvT = np.ascontiguousarray(f32("Wv").T).astype(BF)
    WoT = np.ascontiguousarray(f32("Wo").T).astype(BF)
    bq, bk, bv, bo = f32("bq"), f32("bk"), f32("bv"), f32("bo")

    bq_l = np.ascontiguousarray(bq.reshape(NCH, 128).T)
    bk_l = np.ascontiguousarray(bk.reshape(NCH, 128).T)
    bv_row = bv.reshape(1, DM)
    bo_row = bo.reshape(1, DM)
    ones_row = np.ones((1, 128), np.float32)

    import ml_dtypes
    lj = np.arange(128)[:, None]
    ir = np.arange(512)[None, :]
    mask01 = ((ir >= lj) & (ir <= lj + WIN // 2)).astype(ml_dtypes.bfloat16)

    shared = dict(
        WqT=WqT, WkT=WkT, WvT=WvT, WoT=WoT,
        bq_l=bq_l, bk_l=bk_l, bv_row=bv_row, bo_row=bo_row,
        ones_row=ones_row, mask01=mask01,
    )
    return [
        dict(
            qT=np.ascontiguousarray(q[b].T).astype(BF),
            kT=np.ascontiguousarray(k[b].T).astype(BF),
            vT=np.ascontiguousarray(v[b].T).astype(BF),
            **shared,
        )
        for b in range(B)
    ]


def kernel(**inputs) -> np.ndarray:
    global _NC_CACHE
    if _NC_CACHE is None:
        _NC_CACHE = build_nc()
    in_maps = _host_inputs(inputs)
    res = run_bass_kernel_spmd(_NC_CACHE, in_maps, core_ids=list(range(N_CORES)))
    return np.stack([res.results[b]["out"] for b in range(N_CORES)], axis=0)



# revision 28
# speedup vs baseline: 1.5396x; 1.5396x over previous
"""Trainium2 Bass kernel for nn_MultiHeadCrossAttention_84542136254903.

Sliding-window causal cross-attention (query i attends keys [i-256, i]),
16 heads, d_model 1024. Sharded data-parallel over batch B=8 across the 8
NeuronCores; each core runs the full per-batch-element pipeline:

  q = query @ Wq.T + bq ; k = key @ Wk.T + bk ; v = value @ Wv.T + bv
  S = (q k^T) / 8  (banded: |i-j| window)  ;  P = softmax_masked(S)
  out = (P v) @ Wo.T + bo

Layout strategy (per core):
  - query/key/value and all weights are transposed on the host (cheap numpy
    marshalling, like the batch sharding itself) and converted to bf16, so
    SBUF holds query^T, key^T, value^T, Wq^T, Wk^T, Wv^T, Wo^T via plain
    contiguous DMA loads at half the HBM traffic of fp32. All loads go
    through the two HWDGE queues (sync + scalar), V path first since it
    gates every PV matmul; descriptor processing is ~630ns per DMA so the
    tiny constant loads are emitted behind the V path.
  - Projections run in bf16 (full rate on the PE, FWL weight loads) and
    produce QT=[d_model, Q] and KT=[d_model, T] (feature-major) plus V in
    natural [T, d_model] bf16 with a per-head ones column appended.
  - The middle phase is software-pipelined at head-pair granularity: the
    Q/K projections for pair p+1 are emitted into the exp-wait gaps of pair
    p's score chunks, and pair p-1's PV+normalize is emitted after pair p's
    first chunk, so PE, ACT (exp), DVE and Pool all stay fed. PSUM budget:
    proj pool 2 banks + st 2x2 + ot(region tiles) 2x1 = 8.
  - Attention is computed transposed: for each (head, key-chunk of 128),
    ST[j, i] over the 384-wide query window [j0, j0+384), with the two heads
    of a pair row-packed via tile_position so their K=64 matmuls run
    concurrently in disjoint row-groups of the PE array. exp on ACT ->
    bf16, band mask as a bf16 multiply (split between DVE and Pool engines;
    Pool cannot touch PSUM so SBUF-only work like this is all it can take),
    then bf16 PV matmuls accumulate OT_aug[65, 512] per (head, PSUM region)
    via overlapping-window accumulation (per-2KB-region pending-zero
    semantics). Row 64 (from the ones column of V) is the softmax
    denominator, already in free-dim layout: DVE reciprocal (PSUM row ->
    bf16 SBUF row), a stride-0 DMA replicates it across 64 partitions
    off-engine (cheaper than Pool partition_broadcast), and one fused DVE
    multiply normalizes and evacuates PSUM->SBUF.
  - Out-projection in bf16 reads OT directly (both operands feature-major,
    no transposes anywhere on the PE), stages PSUM->SBUF (DMA cannot read
    PSUM) and DMAs to DRAM.
"""

import os
import numpy as np

import concourse.bass as bass
import concourse.bacc as bacc
import concourse.tile as tile
from concourse import mybir
from concourse.bass_utils import run_bass_kernel_spmd
from concourse.vector_clock import ScopedClock
from contextlib import ExitStack

F32 = mybir.dt.float32
F32R = mybir.dt.float32r
BF16 = mybir.dt.bfloat16
AF = mybir.ActivationFunctionType

B, Q, T = 8, 1024, 1024
DQ, DK, DV, DM, H = 128, 256, 256, 1024, 16
HD = DM // H  # 64
WIN = 512
SCALE = HD ** -0.5
N_CORES = 8
NCH = T // 128  # 8 key chunks / query chunks / m chunks

# matmul dtype for the fp32 stages (projections, scores, out-proj).
MM_DT = F32R

# head-pairs whose band-mask multiply runs on the Pool (gpsimd) engine
# instead of DVE, to balance engine load. (Pool cannot touch PSUM, so the
# mask multiply on SBUF pt tiles is the main work it can absorb.)
POOL_MASK_PAIRS = frozenset({3, 4, 5, 6, 7})


class _TileContextFixed(tile.TileContext):
    """Work around this walrus build's 1-sem-wait-per-CTRL-instruction limit:
    the Tile kernel-tail drain arrives with one wait per outstanding
    semaphore; keep the first on the Drain and chain the rest as single-wait
    nops on the same engine (sequential, so semantics are unchanged)."""

    def _drain_and_barrier(self, tick_clock, wait_clock):
        nc = self.nc
        drain_inst = nc.sync.drain()
        wait_clock.add_sem_waits(
            drain_inst.ins, ScopedClock({None: tick_clock.global_clock})
        )
        si = drain_inst.ins.sync_info
        if si is not None and si.on_wait and len(si.on_wait) > 1:
            waits = list(si.on_wait)
            si.on_wait = [waits[0]]
            drain_inst.ins.sync_info = si
            sem_map = {s.name: s for s in self.sems.allocated().values()}
            for w in waits[1:]:
                sem = sem_map[w.ant_name]
                assert w.wait_mode == "sem-ge-imm", w.wait_mode
                nc.sync.wait_ge(sem, w.wait_value)

        nc.all_engine_barrier()
        assert self.sems is not None
        popped = nc._tile_sem_poison_stack.pop()
        assert popped is self._sem_poison
        nc.clear_and_free_semaphores(list(self.sems.allocated().values()))
        nc.all_engine_barrier()


def _win(c):
    """Query window width for key chunk c (keys [128c, 128c+128)).

    Chunks 0 and 4 are widened to 512 so their PT block fully covers a
    512-column PSUM region of OT_aug: the widened area is entirely masked
    (i - j > 256 there), and it lets the first PV matmul per region open it
    with start=True covering the whole pending-zero region."""
    if c in (0, 4):
        return 512
    return min(384, T - 128 * c)


def build_nc(repeat=1):
    nc = bacc.Bacc(
        "TRN2", target_bir_lowering=False, debug=False, num_devices=N_CORES
    )

    def din(name, shape, dt=F32):
        return nc.dram_tensor(name, shape, dt, kind="ExternalInput").ap()

    qT_d = nc.dram_tensor("qT", [DQ, Q], BF16, kind="ExternalInput").ap()
    kT_d = nc.dram_tensor("kT", [DK, T], BF16, kind="ExternalInput").ap()
    vT_d = nc.dram_tensor("vT", [DV, T], BF16, kind="ExternalInput").ap()
    WqT_d = nc.dram_tensor("WqT", [DQ, DM], BF16, kind="ExternalInput").ap()
    WkT_d = nc.dram_tensor("WkT", [DK, DM], BF16, kind="ExternalInput").ap()
    WvT_d = nc.dram_tensor("WvT", [DV, DM], BF16, kind="ExternalInput").ap()
    WoT_d = nc.dram_tensor("WoT", [DM, DM], BF16, kind="ExternalInput").ap()
    bq_l = din("bq_l", [128, NCH])       # bq_l[p, c] = bq[128c + p]
    bk_l = din("bk_l", [128, NCH])
    bv_row = din("bv_row", [1, DM], F32R)
    bo_row = din("bo_row", [1, DM], F32R)
    ones_row = din("ones_row", [1, 128], F32R)
    mask01 = nc.dram_tensor("mask01", [128, 512], BF16, kind="ExternalInput").ap()

    out = nc.dram_tensor("out", [Q, DM], F32, kind="ExternalOutput").ap()

    with _TileContextFixed(nc) as tc, ExitStack() as ctx:
        small = ctx.enter_context(tc.tile_pool(name="small", bufs=1))
        persist = ctx.enter_context(tc.tile_pool(name="persist", bufs=1))

        # ---- small constant tiles (loads emitted below, after the V path) ----
        bq_t = small.tile([128, NCH], F32, tag="bq")
        bk_t = small.tile([128, NCH], F32, tag="bk")
        bv_t = small.tile([1, DM], F32R, tag="bv")
        bo_t = small.tile([1, DM], F32R, tag="bo")
        ones_t = small.tile([1, 128], F32R, tag="ones")
        mask_t = small.tile([128, 512], BF16, tag="mask")

        # ---- persistent tiles ------------------------------------------------
        qT = persist.tile([128, Q], BF16, tag="qT")          # query^T [DQ, Q]
        WqT = persist.tile([128, DM], BF16, tag="WqT")       # Wq^T [DQ, DM]
        kT = [persist.tile([128, T], BF16, tag=f"kT{i}", name=f"kT{i}") for i in range(2)]
        WkT = [persist.tile([128, DM], BF16, tag=f"WkT{i}", name=f"WkT{i}") for i in range(2)]
        WoT = [persist.tile([128, DM], BF16, tag=f"WoT{i}", name=f"WoT{i}") for i in range(NCH)]
        QT = [persist.tile([128, Q], BF16, tag=f"QT{i}", name=f"QT{i}") for i in range(NCH)]
        KT = [persist.tile([128, T], BF16, tag=f"KT{i}", name=f"KT{i}") for i in range(NCH)]
        # V natural [T, d_model] bf16, 65 columns per head (64 dims + ones)
        Vb = [persist.tile([128, 65 * H], BF16, tag=f"Vb{i}", name=f"Vb{i}") for i in range(NCH)]
        OT = [persist.tile([128, Q], BF16, tag=f"OT{i}", name=f"OT{i}") for i in range(NCH)]

        # ---- loads (V path first: it gates every PV matmul) ------------------
        # All loads go through the two HWDGE queues (sync + scalar); SWDGE
        # (gpsimd et al) burns Pool engine time per descriptor. Order is
        # dependency-critical-path: V path, then K, Q, WoT last.
        vw = ctx.enter_context(tc.tile_pool(name="vw", bufs=1))
        vT = [vw.tile([128, T], BF16, tag=f"vT{i}", name=f"vT{i}") for i in range(2)]
        WvT = [vw.tile([128, DM], BF16, tag=f"WvT{i}", name=f"WvT{i}") for i in range(2)]
        for i in range(2):
            nc.sync.dma_start(vT[i][:], vT_d[128 * i:128 * (i + 1), :])
            nc.scalar.dma_start(WvT[i][:], WvT_d[128 * i:128 * (i + 1), :])
        # consts the V path needs (bias matmul) right after vT on sync;
        # the rest follow on scalar behind WvT.
        nc.sync.dma_start(ones_t[:], ones_row[:])
        nc.sync.dma_start(bv_t[:], bv_row[:])
        nc.scalar.dma_start(mask_t[:], mask01[:])
        nc.scalar.dma_start(bq_t[:], bq_l[:])
        nc.scalar.dma_start(bk_t[:], bk_l[:])
        nc.scalar.dma_start(bo_t[:], bo_row[:])
        for i in range(2):
            nc.sync.dma_start(kT[i][:], kT_d[128 * i:128 * (i + 1), :])
            nc.scalar.dma_start(WkT[i][:], WkT_d[128 * i:128 * (i + 1), :])
        nc.sync.dma_start(qT[:], qT_d[:])
        nc.scalar.dma_start(WqT[:], WqT_d[:])
        for i in range(NCH):
            eng = (nc.scalar, nc.sync)[i % 2]
            eng.dma_start(WoT[i][:], WoT_d[128 * i:128 * (i + 1), :])

        # PV window pieces, split at the 512-col PSUM region boundary.
        # Chunks 0 and 4 have 512-wide windows, so the first piece of each
        # region covers it fully (opens it with start=True).
        pieces = []  # (c, lo, hi, region)
        for c in range(NCH):
            lo, hi = 128 * c, 128 * c + _win(c)
            for b0, b1 in ((0, 512), (512, 1024)):
                ps_, pe_ = max(lo, b0), min(hi, b1)
                if ps_ < pe_:
                    pieces.append((c, ps_, pe_, b0 // 512))
        pieces.sort(key=lambda p: (p[3], p[2] - p[1] != 512, p[1]))
        first_i, last_i = {}, {}
        for idx, (c, ps_, pe_, rg) in enumerate(pieces):
            first_i.setdefault(rg, idx)
            last_i[rg] = idx

        for _rep in range(repeat):
            # ---- V projection (gates every PV matmul) ------------------------
            # Own 4-buf PSUM scope so it runs mm-paced; closes before the
            # pipelined middle opens its pools.
            with tc.tile_pool(name="vproj_psum", bufs=4, space="PSUM") as pjv:
                for jc in range(NCH):
                    js = slice(128 * jc, 128 * (jc + 1))
                    for half in range(2):
                        ps = pjv.tile([128, 512], F32, tag="vp")
                        sl = slice(512 * half, 512 * (half + 1))
                        for cc in range(2):
                            nc.tensor.matmul(
                                ps[:],
                                vT[cc][:, js],
                                WvT[cc][:, sl],
                                start=(cc == 0), stop=False,
                            )
                        nc.tensor.matmul(
                            ps[:], ones_t[:], bv_t[:, sl],
                            start=False, stop=True,
                        )
                        vdst = Vb[jc][:].rearrange("p (h c) -> p h c", c=65)
                        vsrc = ps[:].rearrange("p (h c) -> p h c", c=64)
                        h0 = 8 * half
                        if half == 0:
                            nc.vector.tensor_copy(vdst[:, h0:h0 + 8, 0:64], vsrc[:])
                        else:
                            nc.scalar.activation(
                                vdst[:, h0:h0 + 8, 0:64], vsrc[:], AF.Copy,
                            )
                    ones_col = Vb[jc][:].rearrange("p (h c) -> p h c", c=65)[:, :, 64:65]
                    nc.gpsimd.memset(ones_col, 1.0)

            # PSUM budget for the pipelined middle: proj/outproj pool 2 banks
            # + st 2 banks + ot 4 banks = 8. Pools coexist so the Q/K
            # projections software-pipeline into the attention loop and PE
            # fills exp-wait gaps with the next pair's projections.
            with (
                tc.tile_pool(name="proj_psum", bufs=2, space="PSUM") as pj,
                tc.tile_pool(name="st_psum", bufs=2, space="PSUM") as stp,
                tc.tile_pool(name="ot_psum", bufs=2, space="PSUM") as otp,
                tc.tile_pool(name="pt_sb", bufs=10) as ptp,
                tc.tile_pool(name="recip_sb", bufs=4) as rcp,
            ):

                # Q/K projection for one pair, split into 4 emission steps so
                # they interleave between score chunks of the previous pair.
                # Bias-add evacuation on DVE (ACT must stay exp-only in the
                # pipelined middle phase).
                def kq_steps(hp):
                    def q_half(half):
                        def emit():
                            sl = slice(512 * half, 512 * (half + 1))
                            ps = pj.tile([128, 512], F32, tag="pp")
                            nc.tensor.matmul(
                                ps[:], WqT[:, 128 * hp:128 * (hp + 1)], qT[:, sl],
                                start=True, stop=True,
                            )
                            nc.vector.tensor_scalar_add(
                                QT[hp][:, sl], ps[:], bq_t[:, hp:hp + 1],
                            )
                        return emit
                    def k_half(half):
                        def emit():
                            sl = slice(512 * half, 512 * (half + 1))
                            ps = pj.tile([128, 512], F32, tag="pp")
                            for cc in range(2):
                                nc.tensor.matmul(
                                    ps[:],
                                    WkT[cc][:, 128 * hp:128 * (hp + 1)],
                                    kT[cc][:, sl],
                                    start=(cc == 0), stop=(cc == 1),
                                )
                            nc.vector.tensor_scalar_add(
                                KT[hp][:, sl], ps[:], bk_t[:, hp:hp + 1],
                            )
                        return emit
                    return [q_half(0), k_half(0), q_half(1), k_half(1)]

                def emit_chunk(hp, c, pts):
                    heads = (2 * hp, 2 * hp + 1)
                    W = _win(c)          # PV window (512 for c in {0,4})
                    Wc = min(W, 384)     # live score columns
                    i0 = 128 * c
                    st = stp.tile([128, 1024], F32, tag="st")
                    for h in heads:
                        prow = (h % 2) * 64
                        o = 512 * (h % 2)
                        # tile_position row-packs the two heads' K=64 matmuls
                        # into disjoint row-groups of the PE array so they
                        # run concurrently.
                        nc.tensor.matmul(
                            st[:, o:o + Wc],
                            KT[hp][prow:prow + 64, 128 * c:128 * (c + 1)],
                            QT[hp][prow:prow + 64, i0:i0 + Wc],
                            start=True, stop=True,
                            tile_position=(prow, 0),
                        )
                    pt = ptp.tile([128, 1024], BF16, tag="pt")
                    st3 = st[:].rearrange("p (g f) -> p g f", g=2)
                    pt3 = pt[:].rearrange("p (g f) -> p g f", g=2)
                    nc.scalar.activation(
                        pt3[:, :, 0:Wc], st3[:, :, 0:Wc], AF.Exp,
                        scale=float(SCALE),
                    )
                    # Only the first and third 128-col blocks of each window
                    # are partially masked; one strided multiply covers both
                    # heads. Pool (SBUF-only engine, ~2.3x slower at TT)
                    # takes four chunks, DVE the rest, balancing the middle.
                    eng = nc.gpsimd if c in (0, 1, 2, 4) else nc.vector
                    pt4 = pt[:].rearrange("p (g b f) -> p g b f", g=2, f=128)
                    m3 = mask_t[:].rearrange("p (b f) -> p b f", f=128)
                    if Wc > 256:
                        v = pt4[:, :, 0:3:2, :]
                        m = m3[:, 0:3:2, :]
                    else:
                        v = pt4[:, :, 0:1, :]
                        m = m3[:, 0:1, :]
                    # broadcast the mask across the two head-halves with a
                    # stride-0 free dim
                    m2 = bass.AP(
                        m.tensor, m.offset,
                        [m.ap[0], [0, 2]] + list(m.ap[1:]),
                    )
                    eng.tensor_mul(v, v, m2)
                    if W > Wc:
                        nc.gpsimd.memset(pt3[:, :, Wc:W], 0.0)
                    pts.append(pt)

                def emit_pv_norm(hp, pts):
                    # PV + normalize per (head, 512-col PSUM region): region
                    # tiles are one bank each, so st can keep 2 bufs and the
                    # whole middle fits the 8-bank PSUM budget.
                    heads = (2 * hp, 2 * hp + 1)
                    for rg in (0, 1):
                        for h in heads:
                            prow = (h % 2) * 64
                            o = 512 * (h % 2)
                            ot = otp.tile([65, 512], F32, tag="ot")
                            rp = [p for p in pieces if p[3] == rg]
                            for idx, (c, ps_, pe_, _rg) in enumerate(rp):
                                nc.tensor.matmul(
                                    ot[:, ps_ - 512 * rg:pe_ - 512 * rg],
                                    Vb[c][:, 65 * h:65 * (h + 1)],
                                    pts[c][:, o + ps_ - 128 * c:o + pe_ - 128 * c],
                                    start=(idx == 0),
                                    stop=(idx == len(rp) - 1),
                                    skip_group_check=True,
                                )
                            # normalize: row 64 holds the softmax
                            # denominators. DVE reciprocal (PSUM row -> bf16
                            # SBUF row; reciprocal_approx_fast would be 5x
                            # cheaper but its custom-DVE op miscomputes on HW
                            # through this NEFF path), stride-0 DMA
                            # replicates it across 64 partitions
                            # (off-engine), DVE multiply normalizes and
                            # evacuates PSUM->SBUF.
                            recipT = rcp.tile([1, 512], BF16, tag="recipT")
                            with nc.allow_low_precision(
                                reason="softmax denom recip bf16; tol 2e-2"
                            ):
                                nc.vector.reciprocal(recipT[:], ot[64:65, :])
                            recipB = rcp.tile([64, 512], BF16, tag="recipB")
                            srcT = recipT[:]
                            src0 = bass.AP(
                                srcT.tensor, srcT.offset,
                                [srcT.ap[0], [0, 64]] + list(srcT.ap[1:]),
                            )
                            nc.sync.dma_start(recipB[:], src0)
                            nc.vector.tensor_mul(
                                OT[hp][prow:prow + 64, 512 * rg:512 * (rg + 1)],
                                ot[0:64, :], recipB[:],
                            )

                # ---- pipelined attention middle ------------------------------
                # Per pair: emit chunk 0, then the previous pair's PV+normalize
                # (so PE works while ACT streams this pair's exps), remaining
                # chunks with the next pair's K/Q projection steps interleaved
                # into the exp-wait gaps.
                for step in kq_steps(0):
                    step()
                prev = None
                for hp in range(H // 2):
                    steps = kq_steps(hp + 1) if hp + 1 < H // 2 else []
                    pts = []
                    for c in range(NCH):
                        emit_chunk(hp, c, pts)
                        if c == 0 and prev is not None:
                            emit_pv_norm(*prev)
                        if c in (1, 3, 5, 7) and steps:
                            steps[(c - 1) // 2]()
                    prev = (hp, pts)
                emit_pv_norm(*prev)

            # ---- out projection (own scope: middle PSUM pools are closed,
            # so 4 psum bufs keep PE mm-paced; out DMAs alternate the two
            # HWDGE queues so the drain isn't single-queue bound) ------------
            with (
                tc.tile_pool(name="out_psum", bufs=4, space="PSUM") as op,
                tc.tile_pool(name="out_sb", bufs=6) as osb,
            ):
                for ic in range(NCH):
                    isl = slice(128 * ic, 128 * (ic + 1))
                    for half in range(2):
                        ps = op.tile([128, 512], F32, tag="op")
                        sl = slice(512 * half, 512 * (half + 1))
                        for kc in range(NCH):
                            nc.tensor.matmul(
                                ps[:],
                                OT[kc][:, isl],
                                WoT[kc][:, sl],
                                start=(kc == 0), stop=False,
                            )
                        nc.tensor.matmul(
                            ps[:], ones_t[:], bo_t[:, sl],
                            start=False, stop=True,
                        )
                        st_out = osb.tile([128, 512], F32, tag="ostage")
                        if half == 0:
                            nc.vector.tensor_copy(st_out[:], ps[:])
                        else:
                            nc.scalar.copy(st_out[:], ps[:])
                        eng = (nc.sync, nc.scalar)[(2 * ic + half) % 2]
                        eng.dma_start(out[isl, sl], st_out[:])

    nc.compile()
    return nc


_NC_CACHE = None


def _host_inputs(inputs):
    """Per-core in_maps from the full-batch inputs (host-side transposes)."""
    def f32(name):
        return np.asarray(inputs[name], dtype=np.float32)

    import ml_dtypes as _mld
    BF = _mld.bfloat16
    q, k, v = f32("query"), f32("key"), f32("value")
    WqT = np.ascontiguousarray(f32("Wq").T).astype(BF)
    WkT = np.ascontiguousarray(f32("Wk").T).astype(BF)
    WvT = np.ascontiguousarray(f32("Wv").T).astype(BF)
    WoT = np.ascontiguousarray(f32("Wo").T).astype(BF)
    bq, bk, bv, bo = f32("bq"), f32("bk"), f32("bv"), f32("bo")

    bq_l = np.ascontiguousarray(bq.reshape(NCH, 128).T)
    bk_l = np.ascontiguousarray(bk.reshape(NCH, 128).T)
    bv_row = bv.reshape(1, DM)
    bo_row = bo.reshape(1, DM)
    ones_row = np.ones((1, 128), np.float32)

    import ml_dtypes
    lj = np.arange(128)[:, None]
    ir = np.arange(512)[None, :]
    mask01 = ((ir >= lj) & (ir <= lj + WIN // 2)).astype(ml_dtypes.bfloat16)

    shared = dict(
        WqT=WqT, WkT=WkT, WvT=WvT, WoT=WoT,
        bq_l=bq_l, bk_l=bk_l, bv_row=bv_row, bo_row=bo_row,
        ones_row=ones_row, mask01=mask01,
    )
    return [
        dict(
            qT=np.ascontiguousarray(q[b].T).astype(BF),
            kT=np.ascontiguousarray(k[b].T).astype(BF),
            vT=np.ascontiguousarray(v[b].T).astype(BF),
            **shared,
        )
        for b in range(B)
    ]


def kernel(**inputs) -> np.ndarray:
    global _NC_CACHE
    if _NC_CACHE is None:
        _NC_CACHE = build_nc()
    in_maps = _host_inputs(inputs)
    res = run_bass_kernel_spmd(_NC_CACHE, in_maps, core_ids=list(range(N_CORES)))
    return np.stack([res.results[b]["out"] for b in range(N_CORES)], axis=0)



# revision 31
# speedup vs baseline: 2.1979x; 1.4276x over previous
"""Trainium2 Bass kernel for nn_MultiHeadCrossAttention_84542136254903.

Sliding-window causal cross-attention (query i attends keys [i-256, i]),
16 heads, d_model 1024. Sharded data-parallel over batch B=8 across the 8
NeuronCores; each core runs the full per-batch-element pipeline:

  q = query @ Wq.T + bq ; k = key @ Wk.T + bk ; v = value @ Wv.T + bv
  S = (q k^T) / 8  (banded: |i-j| window)  ;  P = softmax_masked(S)
  out = (P v) @ Wo.T + bo

Layout strategy (per core):
  - query/key/value and all weights are transposed on the host (cheap numpy
    marshalling, like the batch sharding itself) and converted to bf16, so
    SBUF holds query^T, key^T, value^T, Wq^T, Wk^T, Wv^T, Wo^T via plain
    contiguous DMA loads at half the HBM traffic of fp32. All loads go
    through the two HWDGE queues (sync + scalar), V path first since it
    gates every PV matmul; descriptor processing is ~630ns per DMA so the
    tiny constant loads are emitted behind the V path.
  - Projections run in bf16 (full rate on the PE, FWL weight loads) and
    produce QT=[d_model, Q] and KT=[d_model, T] (feature-major) plus V in
    natural [T, d_model] bf16 with a per-head ones column appended.
  - The middle phase is software-pipelined at head-pair granularity: the
    Q/K projections for pair p+1 are emitted into the exp-wait gaps of pair
    p's score chunks, and pair p-1's PV+normalize is emitted after pair p's
    first chunk, so PE, ACT (exp), DVE and Pool all stay fed. PSUM budget:
    proj pool 2 banks + st 2x2 + ot(region tiles) 2x1 = 8.
  - Attention is computed transposed: for each (head, key-chunk of 128),
    ST[j, i] over the 384-wide query window [j0, j0+384), with the two heads
    of a pair row-packed via tile_position so their K=64 matmuls run
    concurrently in disjoint row-groups of the PE array. exp on ACT ->
    bf16, band mask as a bf16 multiply (split between DVE and Pool engines;
    Pool cannot touch PSUM so SBUF-only work like this is all it can take),
    then bf16 PV matmuls accumulate OT_aug[65, 512] per (head, PSUM region)
    via overlapping-window accumulation (per-2KB-region pending-zero
    semantics). Row 64 (from the ones column of V) is the softmax
    denominator, already in free-dim layout: DVE reciprocal (PSUM row ->
    bf16 SBUF row), a stride-0 DMA replicates it across 64 partitions
    off-engine (cheaper than Pool partition_broadcast), and one fused DVE
    multiply normalizes and evacuates PSUM->SBUF.
  - Out-projection in bf16 reads OT directly (both operands feature-major,
    no transposes anywhere on the PE), stages PSUM->SBUF (DMA cannot read
    PSUM) and DMAs to DRAM.
"""

import os
import numpy as np

import concourse.bass as bass
import concourse.bacc as bacc
import concourse.tile as tile
from concourse import mybir
from concourse.bass_utils import run_bass_kernel_spmd
from concourse.vector_clock import ScopedClock
from contextlib import ExitStack

F32 = mybir.dt.float32
F32R = mybir.dt.float32r
BF16 = mybir.dt.bfloat16
AF = mybir.ActivationFunctionType

B, Q, T = 8, 1024, 1024
DQ, DK, DV, DM, H = 128, 256, 256, 1024, 16
HD = DM // H  # 64
WIN = 512
SCALE = HD ** -0.5
N_CORES = 8
NCH = T // 128  # 8 key chunks / query chunks / m chunks

# matmul dtype for the fp32 stages (projections, scores, out-proj).
MM_DT = F32R

# head-pairs whose band-mask multiply runs on the Pool (gpsimd) engine
# instead of DVE, to balance engine load. (Pool cannot touch PSUM, so the
# mask multiply on SBUF pt tiles is the main work it can absorb.)
POOL_MASK_PAIRS = frozenset({3, 4, 5, 6, 7})


class _TileContextFixed(tile.TileContext):
    """Work around this walrus build's 1-sem-wait-per-CTRL-instruction limit:
    the Tile kernel-tail drain arrives with one wait per outstanding
    semaphore; keep the first on the Drain and chain the rest as single-wait
    nops on the same engine (sequential, so semantics are unchanged)."""

    def _drain_and_barrier(self, tick_clock, wait_clock):
        nc = self.nc
        drain_inst = nc.sync.drain()
        wait_clock.add_sem_waits(
            drain_inst.ins, ScopedClock({None: tick_clock.global_clock})
        )
        si = drain_inst.ins.sync_info
        if si is not None and si.on_wait and len(si.on_wait) > 1:
            waits = list(si.on_wait)
            si.on_wait = [waits[0]]
            drain_inst.ins.sync_info = si
            sem_map = {s.name: s for s in self.sems.allocated().values()}
            for w in waits[1:]:
                sem = sem_map[w.ant_name]
                assert w.wait_mode == "sem-ge-imm", w.wait_mode
                nc.sync.wait_ge(sem, w.wait_value)

        nc.all_engine_barrier()
        assert self.sems is not None
        popped = nc._tile_sem_poison_stack.pop()
        assert popped is self._sem_poison
        nc.clear_and_free_semaphores(list(self.sems.allocated().values()))
        nc.all_engine_barrier()


def _win(c):
    """Query window width for key chunk c (keys [128c, 128c+128)).

    Chunks 0 and 4 are widened to 512 so their PT block fully covers a
    512-column PSUM region of OT_aug: the widened area is entirely masked
    (i - j > 256 there), and it lets the first PV matmul per region open it
    with start=True covering the whole pending-zero region."""
    if c in (0, 4):
        return 512
    return min(384, T - 128 * c)


def build_nc(repeat=1):
    nc = bacc.Bacc(
        "TRN2", target_bir_lowering=False, debug=False, num_devices=N_CORES
    )

    def din(name, shape, dt=F32):
        return nc.dram_tensor(name, shape, dt, kind="ExternalInput").ap()

    qT_d = nc.dram_tensor("qT", [DQ, Q], BF16, kind="ExternalInput").ap()
    kT_d = nc.dram_tensor("kT", [DK, T], BF16, kind="ExternalInput").ap()
    vT_d = nc.dram_tensor("vT", [DV, T], BF16, kind="ExternalInput").ap()
    WqT_d = nc.dram_tensor("WqT", [DQ, DM], BF16, kind="ExternalInput").ap()
    WkT_d = nc.dram_tensor("WkT", [DK, DM], BF16, kind="ExternalInput").ap()
    WvT_d = nc.dram_tensor("WvT", [DV, DM], BF16, kind="ExternalInput").ap()
    WoT_d = nc.dram_tensor("WoT", [DM, DM], BF16, kind="ExternalInput").ap()
    bq_l = din("bq_l", [128, NCH])       # bq_l[p, c] = bq[128c + p]
    bk_l = din("bk_l", [128, NCH])
    bv_row = din("bv_row", [1, DM], F32R)
    bo_row = din("bo_row", [1, DM], F32R)
    ones_row = din("ones_row", [1, 128], F32R)
    mask01 = nc.dram_tensor("mask01", [128, 512], BF16, kind="ExternalInput").ap()

    out = nc.dram_tensor("out", [Q, DM], F32, kind="ExternalOutput").ap()

    with _TileContextFixed(nc) as tc, ExitStack() as ctx:
        small = ctx.enter_context(tc.tile_pool(name="small", bufs=1))
        persist = ctx.enter_context(tc.tile_pool(name="persist", bufs=1))

        # ---- small constant tiles (loads emitted below, after the V path) ----
        bq_t = small.tile([128, NCH], F32, tag="bq")
        bk_t = small.tile([128, NCH], F32, tag="bk")
        bv_t = small.tile([1, DM], F32R, tag="bv")
        bo_t = small.tile([1, DM], F32R, tag="bo")
        ones_t = small.tile([1, 128], F32R, tag="ones")
        mask_t = small.tile([128, 512], BF16, tag="mask")

        # ---- persistent tiles ------------------------------------------------
        qT = persist.tile([128, Q], BF16, tag="qT")          # query^T [DQ, Q]
        WqT = persist.tile([128, DM], BF16, tag="WqT")       # Wq^T [DQ, DM]
        kT = [persist.tile([128, T], BF16, tag=f"kT{i}", name=f"kT{i}") for i in range(2)]
        WkT = [persist.tile([128, DM], BF16, tag=f"WkT{i}", name=f"WkT{i}") for i in range(2)]
        WoT = [persist.tile([128, DM], BF16, tag=f"WoT{i}", name=f"WoT{i}") for i in range(NCH)]
        QT = [persist.tile([128, Q], BF16, tag=f"QT{i}", name=f"QT{i}") for i in range(NCH)]
        KT = [persist.tile([128, T], BF16, tag=f"KT{i}", name=f"KT{i}") for i in range(NCH)]
        # V natural [T, d_model] bf16, 65 columns per head (64 dims + ones)
        Vb = [persist.tile([128, 65 * H], BF16, tag=f"Vb{i}", name=f"Vb{i}") for i in range(NCH)]
        OT = [persist.tile([128, Q], BF16, tag=f"OT{i}", name=f"OT{i}") for i in range(NCH)]

        # ---- loads (V path first: it gates every PV matmul) ------------------
        # All loads go through the two HWDGE queues (sync + scalar); SWDGE
        # (gpsimd et al) burns Pool engine time per descriptor. Order is
        # dependency-critical-path: V path, then K, Q, WoT last.
        vw = ctx.enter_context(tc.tile_pool(name="vw", bufs=1))
        vT = [vw.tile([128, T], BF16, tag=f"vT{i}", name=f"vT{i}") for i in range(2)]
        WvT = [vw.tile([128, DM], BF16, tag=f"WvT{i}", name=f"WvT{i}") for i in range(2)]
        for i in range(2):
            nc.sync.dma_start(vT[i][:], vT_d[128 * i:128 * (i + 1), :])
            nc.scalar.dma_start(WvT[i][:], WvT_d[128 * i:128 * (i + 1), :])
        # consts the V path needs (bias matmul) right after vT on sync;
        # the rest follow on scalar behind WvT.
        nc.sync.dma_start(ones_t[:], ones_row[:])
        nc.sync.dma_start(bv_t[:], bv_row[:])
        nc.scalar.dma_start(mask_t[:], mask01[:])
        nc.scalar.dma_start(bq_t[:], bq_l[:])
        nc.scalar.dma_start(bk_t[:], bk_l[:])
        nc.scalar.dma_start(bo_t[:], bo_row[:])
        for i in range(2):
            nc.sync.dma_start(kT[i][:], kT_d[128 * i:128 * (i + 1), :])
            nc.scalar.dma_start(WkT[i][:], WkT_d[128 * i:128 * (i + 1), :])
        nc.sync.dma_start(qT[:], qT_d[:])
        nc.scalar.dma_start(WqT[:], WqT_d[:])
        for i in range(NCH):
            eng = (nc.scalar, nc.sync)[i % 2]
            eng.dma_start(WoT[i][:], WoT_d[128 * i:128 * (i + 1), :])

        # PV window pieces, split at the 512-col PSUM region boundary.
        # Chunks 0 and 4 have 512-wide windows, so the first piece of each
        # region covers it fully (opens it with start=True).
        pieces = []  # (c, lo, hi, region)
        for c in range(NCH):
            lo, hi = 128 * c, 128 * c + _win(c)
            for b0, b1 in ((0, 512), (512, 1024)):
                ps_, pe_ = max(lo, b0), min(hi, b1)
                if ps_ < pe_:
                    pieces.append((c, ps_, pe_, b0 // 512))
        pieces.sort(key=lambda p: (p[3], p[2] - p[1] != 512, p[1]))
        first_i, last_i = {}, {}
        for idx, (c, ps_, pe_, rg) in enumerate(pieces):
            first_i.setdefault(rg, idx)
            last_i[rg] = idx

        for _rep in range(repeat):
            # ---- V projection (gates every PV matmul) ------------------------
            # Own 4-buf PSUM scope so it runs mm-paced; closes before the
            # pipelined middle opens its pools.
            with tc.tile_pool(name="vproj_psum", bufs=4, space="PSUM") as pjv:
                for jc in range(NCH):
                    js = slice(128 * jc, 128 * (jc + 1))
                    for half in range(2):
                        ps = pjv.tile([128, 512], F32, tag="vp")
                        sl = slice(512 * half, 512 * (half + 1))
                        for cc in range(2):
                            nc.tensor.matmul(
                                ps[:],
                                vT[cc][:, js],
                                WvT[cc][:, sl],
                                start=(cc == 0), stop=False,
                            )
                        nc.tensor.matmul(
                            ps[:], ones_t[:], bv_t[:, sl],
                            start=False, stop=True,
                        )
                        vdst = Vb[jc][:].rearrange("p (h c) -> p h c", c=65)
                        vsrc = ps[:].rearrange("p (h c) -> p h c", c=64)
                        h0 = 8 * half
                        if half == 0:
                            nc.vector.tensor_copy(vdst[:, h0:h0 + 8, 0:64], vsrc[:])
                        else:
                            nc.scalar.activation(
                                vdst[:, h0:h0 + 8, 0:64], vsrc[:], AF.Copy,
                            )
                    ones_col = Vb[jc][:].rearrange("p (h c) -> p h c", c=65)[:, :, 64:65]
                    nc.gpsimd.memset(ones_col, 1.0)

            # PSUM budget for the pipelined middle: proj/outproj pool 2 banks
            # + st 2 banks + ot 4 banks = 8. Pools coexist so the Q/K
            # projections software-pipeline into the attention loop and PE
            # fills exp-wait gaps with the next pair's projections.
            with (
                tc.tile_pool(name="proj_psum", bufs=1, space="PSUM") as pj,
                tc.tile_pool(name="st_psum", bufs=2, space="PSUM") as stp,
                tc.tile_pool(name="ot_psum", bufs=3, space="PSUM") as otp,
                tc.tile_pool(name="pt_sb", bufs=10) as ptp,
                tc.tile_pool(name="recip_sb", bufs=4) as rcp,
            ):

                # Q/K projection for one pair, split into 4 emission steps so
                # they interleave between score chunks of the previous pair.
                # Bias-add evacuation on DVE (ACT must stay exp-only in the
                # pipelined middle phase).
                def kq_steps(hp):
                    def q_half(half):
                        def emit():
                            sl = slice(512 * half, 512 * (half + 1))
                            ps = pj.tile([128, 512], F32, tag="pp")
                            nc.tensor.matmul(
                                ps[:], WqT[:, 128 * hp:128 * (hp + 1)], qT[:, sl],
                                start=True, stop=True,
                            )
                            nc.vector.tensor_scalar_add(
                                QT[hp][:, sl], ps[:], bq_t[:, hp:hp + 1],
                            )
                        return emit
                    def k_half(half):
                        def emit():
                            sl = slice(512 * half, 512 * (half + 1))
                            ps = pj.tile([128, 512], F32, tag="pp")
                            for cc in range(2):
                                nc.tensor.matmul(
                                    ps[:],
                                    WkT[cc][:, 128 * hp:128 * (hp + 1)],
                                    kT[cc][:, sl],
                                    start=(cc == 0), stop=(cc == 1),
                                )
                            nc.vector.tensor_scalar_add(
                                KT[hp][:, sl], ps[:], bk_t[:, hp:hp + 1],
                            )
                        return emit
                    return [q_half(0), k_half(0), q_half(1), k_half(1)]

                def emit_chunk(hp, c, pts):
                    heads = (2 * hp, 2 * hp + 1)
                    W = _win(c)          # PV window (512 for c in {0,4})
                    Wc = min(W, 384)     # live score columns
                    i0 = 128 * c
                    st = stp.tile([128, 1024], F32, tag="st")
                    for h in heads:
                        prow = (h % 2) * 64
                        o = 512 * (h % 2)
                        # tile_position row-packs the two heads' K=64 matmuls
                        # into disjoint row-groups of the PE array so they
                        # run concurrently.
                        nc.tensor.matmul(
                            st[:, o:o + Wc],
                            KT[hp][prow:prow + 64, 128 * c:128 * (c + 1)],
                            QT[hp][prow:prow + 64, i0:i0 + Wc],
                            start=True, stop=True,
                            tile_position=(prow, 0),
                        )
                    pt = ptp.tile([128, 1024], BF16, tag="pt")
                    st3 = st[:].rearrange("p (g f) -> p g f", g=2)
                    pt3 = pt[:].rearrange("p (g f) -> p g f", g=2)
                    nc.scalar.activation(
                        pt3[:, :, 0:Wc], st3[:, :, 0:Wc], AF.Exp,
                        scale=float(SCALE),
                    )
                    # Only the first and third 128-col blocks of each window
                    # are partially masked; one strided multiply covers both
                    # heads. Pool (SBUF-only engine, ~2.3x slower at TT)
                    # takes four chunks, DVE the rest, balancing the middle.
                    eng = nc.gpsimd if c in (0, 1, 2, 4) else nc.vector
                    pt4 = pt[:].rearrange("p (g b f) -> p g b f", g=2, f=128)
                    m3 = mask_t[:].rearrange("p (b f) -> p b f", f=128)
                    if Wc > 256:
                        v = pt4[:, :, 0:3:2, :]
                        m = m3[:, 0:3:2, :]
                    else:
                        v = pt4[:, :, 0:1, :]
                        m = m3[:, 0:1, :]
                    # broadcast the mask across the two head-halves with a
                    # stride-0 free dim
                    m2 = bass.AP(
                        m.tensor, m.offset,
                        [m.ap[0], [0, 2]] + list(m.ap[1:]),
                    )
                    eng.tensor_mul(v, v, m2)
                    if W > Wc:
                        nc.gpsimd.memset(pt3[:, :, Wc:W], 0.0)
                    pts.append(pt)

                def emit_pv_norm(hp, pts):
                    # PV + normalize per (head, 512-col PSUM region): region
                    # tiles are one bank each, so st can keep 2 bufs and the
                    # whole middle fits the 8-bank PSUM budget.
                    heads = (2 * hp, 2 * hp + 1)
                    for rg in (0, 1):
                        for h in heads:
                            prow = (h % 2) * 64
                            o = 512 * (h % 2)
                            ot = otp.tile([65, 512], F32, tag="ot")
                            rp = [p for p in pieces if p[3] == rg]
                            for idx, (c, ps_, pe_, _rg) in enumerate(rp):
                                nc.tensor.matmul(
                                    ot[:, ps_ - 512 * rg:pe_ - 512 * rg],
                                    Vb[c][:, 65 * h:65 * (h + 1)],
                                    pts[c][:, o + ps_ - 128 * c:o + pe_ - 128 * c],
                                    start=(idx == 0),
                                    stop=(idx == len(rp) - 1),
                                    skip_group_check=True,
                                )
                            # normalize: row 64 holds the softmax
                            # denominators. DVE reciprocal (PSUM row -> bf16
                            # SBUF row; TT-divide would fuse this but the
                            # ISA has no TT divide, and reciprocal_approx_
                            # fast miscomputes on HW), stride-0 DMA
                            # replicates it across 64 partitions
                            # (off-engine, alternating HWDGE queues), DVE
                            # multiply normalizes and evacuates PSUM->SBUF.
                            recipT = rcp.tile([1, 512], BF16, tag="recipT")
                            with nc.allow_low_precision(
                                reason="softmax denom recip bf16; tol 2e-2"
                            ):
                                nc.vector.reciprocal(recipT[:], ot[64:65, :])
                            recipB = rcp.tile([64, 512], BF16, tag="recipB")
                            srcT = recipT[:]
                            src0 = bass.AP(
                                srcT.tensor, srcT.offset,
                                [srcT.ap[0], [0, 64]] + list(srcT.ap[1:]),
                            )
                            deng = (nc.sync, nc.scalar)[(2 * rg + h) % 2]
                            deng.dma_start(recipB[:], src0)
                            nc.vector.tensor_mul(
                                OT[hp][prow:prow + 64, 512 * rg:512 * (rg + 1)],
                                ot[0:64, :], recipB[:],
                            )

                # ---- pipelined attention middle ------------------------------
                # Per pair: emit chunk 0, then the previous pair's PV+normalize
                # (so PE works while ACT streams this pair's exps), remaining
                # chunks with the next pair's K/Q projection steps interleaved
                # into the exp-wait gaps.
                for step in kq_steps(0):
                    step()
                prev = None
                for hp in range(H // 2):
                    steps = kq_steps(hp + 1) if hp + 1 < H // 2 else []
                    pts = []
                    for c in range(NCH):
                        emit_chunk(hp, c, pts)
                        if c == 0 and prev is not None:
                            emit_pv_norm(*prev)
                        if c in (1, 3, 5, 7) and steps:
                            steps[(c - 1) // 2]()
                    prev = (hp, pts)
                emit_pv_norm(*prev)

            # ---- out projection (own scope: middle PSUM pools are closed,
            # so 4 psum bufs keep PE mm-paced; out DMAs alternate the two
            # HWDGE queues so the drain isn't single-queue bound) ------------
            with (
                tc.tile_pool(name="out_psum", bufs=4, space="PSUM") as op,
                tc.tile_pool(name="out_sb", bufs=6) as osb,
            ):
                for ic in range(NCH):
                    isl = slice(128 * ic, 128 * (ic + 1))
                    for half in range(2):
                        ps = op.tile([128, 512], F32, tag="op")
                        sl = slice(512 * half, 512 * (half + 1))
                        for kc in range(NCH):
                            nc.tensor.matmul(
                                ps[:],
                                OT[kc][:, isl],
                                WoT[kc][:, sl],
                                start=(kc == 0), stop=False,
                            )
                        nc.tensor.matmul(
                            ps[:], ones_t[:], bo_t[:, sl],
                            start=False, stop=True,
                        )
                        st_out = osb.tile([128, 512], F32, tag="ostage")
                        if half == 0:
                            nc.vector.tensor_copy(st_out[:], ps[:])
                        else:
                            nc.scalar.copy(st_out[:], ps[:])
                        eng = (nc.sync, nc.scalar)[(2 * ic + half) % 2]
                        eng.dma_start(out[isl, sl], st_out[:])

    nc.compile()
    return nc


_NC_CACHE = None


def _host_inputs(inputs):
    """Per-core in_maps from the full-batch inputs (host-side transposes)."""
    def f32(name):
        return np.asarray(inputs[name], dtype=np.float32)

    import ml_dtypes as _mld
    BF = _mld.bfloat16
    q, k, v = f32("query"), f32("key"), f32("value")
    WqT = np.ascontiguousarray(f32("Wq").T).astype(BF)
    WkT = np.ascontiguousarray(f32("Wk").T).astype(BF)
    WvT = np.ascontiguousarray(f32("Wv").T).astype(BF)
    WoT = np.ascontiguousarray(f32("Wo").T).astype(BF)
    bq, bk, bv, bo = f32("bq"), f32("bk"), f32("bv"), f32("bo")

    bq_l = np.ascontiguousarray(bq.reshape(NCH, 128).T)
    bk_l = np.ascontiguousarray(bk.reshape(NCH, 128).T)
    bv_row = bv.reshape(1, DM)
    bo_row = bo.reshape(1, DM)
    ones_row = np.ones((1, 128), np.float32)

    import ml_dtypes
    lj = np.arange(128)[:, None]
    ir = np.arange(512)[None, :]
    mask01 = ((ir >= lj) & (ir <= lj + WIN // 2)).astype(ml_dtypes.bfloat16)

    shared = dict(
        WqT=WqT, WkT=WkT, WvT=WvT, WoT=WoT,
        bq_l=bq_l, bk_l=bk_l, bv_row=bv_row, bo_row=bo_row,
        ones_row=ones_row, mask01=mask01,
    )
    return [
        dict(
            qT=np.ascontiguousarray(q[b].T).astype(BF),
            kT=np.ascontiguousarray(k[b].T).astype(BF),
            vT=np.ascontiguousarray(v[b].T).astype(BF),
            **shared,
        )
        for b in range(B)
    ]


def kernel(**inputs) -> np.ndarray:
    global _NC_CACHE
    if _NC_CACHE is None:
        _NC_CACHE = build_nc()
    in_maps = _host_inputs(inputs)
    res = run_bass_kernel_spmd(_NC_CACHE, in_maps, core_ids=list(range(N_CORES)))
    return np.stack([res.results[b]["out"] for b in range(N_CORES)], axis=0)

